# revision 8
# baseline (speedup 1.0000x reference)
"""Single-head attention (B=8, N=2048, D=512, fp32) on 8 TRN2 NeuronCores.

Data-parallel over batch: core i handles batch element i end-to-end.

Algebra: softmax_m(Q_n . K_m) with Q = xWq+bq, K = xWk+bk satisfies
  Q_n . K_m = x_n A x_m^T + x_n u + x_m v + bq.bk,   A = Wq Wk^T,
  u = Wq bk, v = Wk bq.
The (x_n u + bq.bk) term is constant along the softmax axis m and drops.
So with Y = x A + v^T (row-broadcast), scores-equivalent S[n,m] = Y_n . x_m.

Per-core pipeline (all matmuls fp32r = full-rate PE):
  WqT, WkT  <- PE transpose of Wq, Wk            (warms PE during x DMA)
  A[d,d']   = sum_k Wq[d,k] Wk[d',k]             (lhsT=WqT, rhs=WkT)
  v         = Wk bq
  xT        <- PE transpose of x tiles            (d on partitions)
  V         = x Wv + bv                           (natural layout, seq on part)
  YT[d',n]  = sum_d A[d,d'] xT[d,n] + v[d']       (lhsT=A natural, rhs=xT)
  per 512-wide q strip, per key tile kt (software pipelined):
     S^T tile [m=128, q=512] = xT-chunk^T @ YT    (accum over d chunks)
     e = exp(S^T / sqrt(D))                        (ACT, fused scale)
     sums[q-sub, kt] = e[:,sub]^T @ ones           (single tiny matmuls)
     O[q-sub] += e[:,sub]^T @ V[kt]                (PSUM accum over kt ->
                                                    output in NATURAL layout)
  r = 1/rowsum ; O *= r (per-partition scalar) ; DMA out (no transposes)
"""

import numpy as np

import concourse.bass as bass
import concourse.tile as tile
from concourse import bacc, mybir
from concourse import bass_utils
from concourse.bass import ts
from concourse.masks import make_identity
from contextlib import ExitStack

B, N, D = 8, 2048, 512
P = 128
NT = N // P      # 16 seq tiles
DC = D // P      # 4 d chunks
QS = 512         # q-strip width (one PSUM bank of fp32)
NS = N // QS     # 4 strips
SOFTMAX_SCALE = 1.0 / float(np.sqrt(D))

F32 = mybir.dt.float32
F32R = mybir.dt.float32r
BF16 = mybir.dt.bfloat16
AF = mybir.ActivationFunctionType


def _build():
    nc = bacc.Bacc("TRN2", target_bir_lowering=False, debug=False)

    x = nc.dram_tensor("x", [N, D], F32, kind="ExternalInput").ap()
    wq = nc.dram_tensor("wq", [D, D], F32, kind="ExternalInput").ap()
    bq = nc.dram_tensor("bq", [D], F32, kind="ExternalInput").ap()
    wk = nc.dram_tensor("wk", [D, D], F32, kind="ExternalInput").ap()
    bk = nc.dram_tensor("bk", [D], F32, kind="ExternalInput").ap()  # noqa: F841 (drops out of softmax)
    wv = nc.dram_tensor("wv", [D, D], F32, kind="ExternalInput").ap()
    bv = nc.dram_tensor("bv", [D], F32, kind="ExternalInput").ap()
    out = nc.dram_tensor("out", [N, D], F32, kind="ExternalOutput").ap()

    with ExitStack() as ctx:
        tc = ctx.enter_context(tile.TileContext(nc))

        const = ctx.enter_context(tc.tile_pool(name="const", bufs=1))
        io512 = ctx.enter_context(tc.tile_pool(name="io512", bufs=4))
        wstage = ctx.enter_context(tc.tile_pool(name="wstage", bufs=3))
        big = ctx.enter_context(tc.tile_pool(name="big", bufs=1))
        epool = ctx.enter_context(tc.tile_pool(name="epool", bufs=3))
        rpool = ctx.enter_context(tc.tile_pool(name="rpool", bufs=2))
        opool = ctx.enter_context(tc.tile_pool(name="opool", bufs=6))
        ps512 = ctx.enter_context(tc.tile_pool(name="ps512", bufs=2, space="PSUM"))

        # ---- constants ----
        ident = const.tile([P, P], F32)
        make_identity(nc, ident)
        ones_f = const.tile([P, 2], F32)
        nc.vector.memset(ones_f, 0.5)
        warm = const.tile([P, 2], F32)
        nc.scalar.activation(warm[:], ones_f[:], AF.Exp)
        ones_r = const.tile([P, 2], BF16)
        nc.vector.tensor_copy(out=ones_r[:], in_=ones_f[:])
        # ---- big persistent tensors ----
        xT = big.tile([P, DC, N], BF16)     # x^T: d on partitions
        YT = big.tile([P, DC, N], BF16)     # (xA + v)^T: d' on partitions
        V = big.tile([P, NT, D], BF16)      # natural: seq on partitions
        A_sb = big.tile([P, DC, D], BF16)   # A[d, d'], d = ki*128+p
        wqT = big.tile([P, DC, D], BF16)    # WqT[k, d], k = kc*128+p
        wkT = big.tile([P, DC, D], BF16)
        v_sb = const.tile([P, DC], F32)     # v = Wk bq, chunked like bq_col

        # ---- phase 0: weights (sync queue), x tiles (scalar queue) ----
        wstg = {}
        for name, wap in (("q", wq), ("k", wk), ("v", wv)):
            wst = wstage.tile([P, DC, D], F32, tag="wstage", name=f"wst_{name}")
            wre = wap.rearrange("(dc p) k -> p dc k", p=P)
            for dc in range(DC):
                nc.sync.dma_start(wst[:, dc, :], wre[:, dc, :])
            wstg[name] = wst
        wv_st = wstg["v"]
        bv_rep = const.tile([P, D], F32)
        nc.sync.dma_start(bv_rep[:], bv[None, :].to_broadcast((P, D)))
        bq_col = const.tile([P, DC], F32)
        nc.sync.dma_start(bq_col[:], bq.rearrange("(c p) -> p c", p=P))
        bq_col_r = const.tile([P, DC, 2], BF16)
        nc.vector.tensor_copy(out=bq_col_r[:, :, 0], in_=bq_col[:])
        nc.vector.tensor_copy(out=bq_col_r[:, :, 1], in_=bq_col[:])

        with tc.tile_pool(name="ps_tr", bufs=2, space="PSUM") as ps_tr:

            def emit_x_tile(t):
                x_t = io512.tile([P, D], F32, tag="io512")
                nc.scalar.dma_start(x_t[:], x[ts(t, P), :])
                tp = ps_tr.tile([P, DC, P], F32, tag="tr")
                for c in range(DC):
                    nc.tensor.transpose(tp[:, c, :], x_t[:, ts(c, P)], ident)
                # single strided copy into xT (ACT casts fp32 psum -> bf16)
                nc.scalar.copy(xT[:, :, ts(t, P)], tp[:])

            # x tiles 0/1 arrive on the scalar queue before most weight
            # chunks land on sync - transpose them first so PE isn't idle
            emit_x_tile(0)
            emit_x_tile(1)

            # transposes: wT[k, d] = W[d, k]; 4 blocks batched per PSUM bank,
            # one strided copy out (dst view [P, kc, 128] at fixed dc)
            for name, wT in (("q", wqT), ("k", wkT)):
                wst = wstg[name]
                for dc in range(DC):
                    tp = ps_tr.tile([P, DC, P], F32, tag="tr")
                    for kc in range(DC):
                        nc.tensor.transpose(tp[:, kc, :], wst[:, dc, ts(kc, P)],
                                            ident)
                    nc.vector.tensor_copy(out=wT[:, :, ts(dc, P)], in_=tp[:])

            def emit_A_v():
                # A[d-chunk dc, d'] = sum_k Wq[d, k] Wk[d', k]
                for dc in range(DC):
                    pa = ps512.tile([P, QS], F32, tag="mm512", name=f"pa_{dc}")
                    for kc in range(DC):
                        nc.tensor.matmul(
                            pa[:], wqT[:, kc, ts(dc, P)], wkT[:, kc, :],
                            start=(kc == 0), stop=(kc == DC - 1),
                        )
                    nc.vector.tensor_copy(out=A_sb[:, dc, :], in_=pa[:])
                # v = Wk @ bq (sequential accumulation groups, one per column)
                pv_ps = ps_tr.tile([P, DC, 2], F32, tag="tr", name="v_ps")
                for dc in range(DC):
                    for jc in range(DC):
                        nc.tensor.matmul(
                            pv_ps[:, dc, :], wkT[:, jc, ts(dc, P)],
                            bq_col_r[:, jc, :],
                            start=(jc == 0), stop=(jc == DC - 1),
                            skip_group_check=True,
                        )
                nc.vector.tensor_copy(out=v_sb[:], in_=pv_ps[:, :, 0])

            wv_r = big.tile([P, DC, D], BF16)
            for dc in range(DC):
                nc.vector.tensor_copy(out=wv_r[:, dc, :], in_=wv_st[:, dc, :])

            # ---- phase 1: x tiles -> xT (PE transpose); V; YT per strip ----
            # Software-pipelined: V(t) and YT(s) are emitted one x-tile late so
            # the PE never head-of-line blocks on the PSUM->SBUF copy of xT.
            def emit_v(t):
                pv = ps512.tile([P, QS], F32, tag="mm512", name=f"pv_{t}")
                for ki in range(DC):
                    nc.tensor.matmul(
                        pv[:], xT[:, ki, ts(t, P)], wv_r[:, ki, :],
                        start=(ki == 0), stop=(ki == DC - 1),
                    )
                nc.vector.tensor_add(out=V[:, t, :], in0=pv[:], in1=bv_rep[:])

            def emit_yt(s):
                for co in range(DC):
                    py = ps512.tile([P, QS], F32, tag="mm512",
                                    name=f"py_{s}_{co}")
                    for ki in range(DC):
                        nc.tensor.matmul(
                            py[:], A_sb[:, ki, ts(co, P)],
                            xT[:, ki, ts(s, QS)],
                            start=(ki == 0), stop=(ki == DC - 1),
                        )
                    nc.scalar.activation(
                        YT[:, co, ts(s, QS)], py[:], AF.Identity,
                        bias=v_sb[:, co:co + 1],
                    )

            for t in range(NT):
                if t >= 2:
                    emit_x_tile(t)
                if t == 6:
                    emit_A_v()
                if t >= 1:
                    emit_v(t - 1)
                if t in (8, 10, 12):
                    emit_yt((t - 8) // 2)
            emit_v(NT - 1)
            emit_yt(NS - 1)

        # ---- phase 2: attention, one 512-wide q strip at a time ----
        with tc.tile_pool(name="ps_o", bufs=4, space="PSUM") as ps_o, \
             tc.tile_pool(name="ps_sums", bufs=2, space="PSUM") as ps_sums:
            # q-strips: three 512-wide, then two 256-wide so the exposed
            # epilogue after the very last matmul covers only 2 output tiles
            strips = [(0, 4), (4, 4), (8, 4), (12, 2), (14, 2)]
            for si, (q0, nsub) in enumerate(strips):
                qw = nsub * P
                sums_ps = ps_sums.tile([P, DC, NT, 2], F32, tag="sums")
                o_ps = [ps_o.tile([P, QS], F32, tag="o", name=f"o_{si}_{c}")
                        for c in range(nsub)]

                def emit_mo(kt, e):
                    for sub in range(nsub):
                        nc.tensor.matmul(
                            sums_ps[:, sub, kt, :], e[:, ts(sub, P)],
                            ones_r[:], start=True, stop=True,
                        )
                        nc.tensor.matmul(
                            o_ps[sub][:], e[:, ts(sub, P)], V[:, kt, :],
                            start=(kt == 0), stop=(kt == NT - 1),
                            skip_group_check=True,
                        )

                pending = None
                for kt in range(NT):
                    st = ps512.tile([P, QS], F32, tag="mm512",
                                    name=f"st_{si}_{kt}")
                    for ki in range(DC):
                        nc.tensor.matmul(
                            st[:, :qw], xT[:, ki, ts(kt, P)],
                            YT[:, ki, q0 * P:q0 * P + qw],
                            start=(ki == 0), stop=(ki == DC - 1),
                        )
                    e = epool.tile([P, QS], BF16, tag="e")
                    nc.scalar.activation(e[:, :qw], st[:, :qw], AF.Exp,
                                         scale=SOFTMAX_SCALE)
                    if pending is not None:
                        emit_mo(*pending)
                    pending = (kt, e)
                emit_mo(*pending)

                # normalize in natural layout: per-partition scalar multiply
                sums_red = rpool.tile([P, DC], F32, tag="sred")
                nc.vector.tensor_reduce(
                    sums_red[:, :nsub], sums_ps[:, :nsub], axis=mybir.AxisListType.XY,
                    op=mybir.AluOpType.add,
                )
                r = rpool.tile([P, DC], F32, tag="r")
                nc.vector.reciprocal(r[:, :nsub], sums_red[:, :nsub])
                for sub in range(nsub):
                    o_sb = opool.tile([P, D], F32, tag="osb")
                    if sub % 2 == 0:
                        nc.vector.tensor_scalar_mul(o_sb[:], o_ps[sub][:],
                                                    r[:, sub:sub + 1])
                    else:
                        nc.scalar.mul(o_sb[:], o_ps[sub][:],
                                      r[:, sub:sub + 1])
                    dma_eng = nc.sync if sub % 2 == 0 else nc.scalar
                    dma_eng.dma_start(out[ts(q0 + sub, P), :], o_sb[:])

    nc.compile()
    return nc


_CACHE = {}


def _get_nc():
    if "nc" not in _CACHE:
        _CACHE["nc"] = _build()
    return _CACHE["nc"]


def kernel(x, Wq_w, Wq_b, Wk_w, Wk_b, Wv_w, Wv_b, _trace=False, _tmpdir=None):
    nc = _get_nc()
    x = np.ascontiguousarray(np.asarray(x, dtype=np.float32))
    args = {
        "wq": Wq_w, "bq": Wq_b,
        "wk": Wk_w, "bk": Wk_b,
        "wv": Wv_w, "bv": Wv_b,
    }
    args = {k: np.ascontiguousarray(np.asarray(v, dtype=np.float32))
            for k, v in args.items()}
    in_maps = [dict(args, x=x[i]) for i in range(B)]
    res = bass_utils.run_bass_kernel_spmd(
        nc, in_maps, core_ids=list(range(B)),
        trace=_trace, tmpdir=_tmpdir,
    )
    out = np.stack([r["out"] for r in res.results], axis=0)
    if _trace:
        kernel.last_results = res
    return out


if __name__ == "__main__":
    rng = np.random.default_rng(0)
    inputs = {
        "x": rng.standard_normal((B, N, D)).astype(np.float32),
        "Wq_w": (0.02 * rng.standard_normal((D, D))).astype(np.float32),
        "Wq_b": np.zeros(D, np.float32),
        "Wk_w": (0.02 * rng.standard_normal((D, D))).astype(np.float32),
        "Wk_b": np.zeros(D, np.float32),
        "Wv_w": (0.02 * rng.standard_normal((D, D))).astype(np.float32),
        "Wv_b": np.zeros(D, np.float32),
    }
    got = kernel(**inputs)
    print("out shape:", got.shape, got.dtype)


# revision 9
# speedup vs baseline: 1.0004x; 1.0004x over previous
"""Single-head attention (B=8, N=2048, D=512, fp32) on 8 TRN2 NeuronCores.

Data-parallel over batch: core i handles batch element i end-to-end.

Algebra: softmax_m(Q_n . K_m) with Q = xWq+bq, K = xWk+bk satisfies
  Q_n . K_m = x_n A x_m^T + x_n u + x_m v + bq.bk,   A = Wq Wk^T,
  u = Wq bk, v = Wk bq.
The (x_n u + bq.bk) term is constant along the softmax axis m and drops.
So with Y = x A + v^T (row-broadcast), scores-equivalent S[n,m] = Y_n . x_m.

Per-core pipeline (all matmuls fp32r = full-rate PE):
  WqT, WkT  <- PE transpose of Wq, Wk            (warms PE during x DMA)
  A[d,d']   = sum_k Wq[d,k] Wk[d',k]             (lhsT=WqT, rhs=WkT)
  v         = Wk bq
  xT        <- PE transpose of x tiles            (d on partitions)
  V         = x Wv + bv                           (natural layout, seq on part)
  YT[d',n]  = sum_d A[d,d'] xT[d,n] + v[d']       (lhsT=A natural, rhs=xT)
  per 512-wide q strip, per key tile kt (software pipelined):
     S^T tile [m=128, q=512] = xT-chunk^T @ YT    (accum over d chunks)
     e = exp(S^T / sqrt(D))                        (ACT, fused scale)
     sums[q-sub, kt] = e[:,sub]^T @ ones           (single tiny matmuls)
     O[q-sub] += e[:,sub]^T @ V[kt]                (PSUM accum over kt ->
                                                    output in NATURAL layout)
  r = 1/rowsum ; O *= r (per-partition scalar) ; DMA out (no transposes)
"""

import numpy as np

import concourse.bass as bass
import concourse.tile as tile
from concourse import bacc, mybir
from concourse import bass_utils
from concourse.bass import ts
from concourse.masks import make_identity
from contextlib import ExitStack

B, N, D = 8, 2048, 512
P = 128
NT = N // P      # 16 seq tiles
DC = D // P      # 4 d chunks
QS = 512         # q-strip width (one PSUM bank of fp32)
NS = N // QS     # 4 strips
SOFTMAX_SCALE = 1.0 / float(np.sqrt(D))

F32 = mybir.dt.float32
F32R = mybir.dt.float32r
BF16 = mybir.dt.bfloat16
AF = mybir.ActivationFunctionType


def _build():
    nc = bacc.Bacc("TRN2", target_bir_lowering=False, debug=False)

    x = nc.dram_tensor("x", [N, D], F32, kind="ExternalInput").ap()
    wq = nc.dram_tensor("wq", [D, D], F32, kind="ExternalInput").ap()
    bq = nc.dram_tensor("bq", [D], F32, kind="ExternalInput").ap()
    wk = nc.dram_tensor("wk", [D, D], F32, kind="ExternalInput").ap()
    bk = nc.dram_tensor("bk", [D], F32, kind="ExternalInput").ap()  # noqa: F841 (drops out of softmax)
    wv = nc.dram_tensor("wv", [D, D], F32, kind="ExternalInput").ap()
    bv = nc.dram_tensor("bv", [D], F32, kind="ExternalInput").ap()
    out = nc.dram_tensor("out", [N, D], F32, kind="ExternalOutput").ap()

    with ExitStack() as ctx:
        tc = ctx.enter_context(tile.TileContext(nc))

        const = ctx.enter_context(tc.tile_pool(name="const", bufs=1))
        io512 = ctx.enter_context(tc.tile_pool(name="io512", bufs=4))
        wstage = ctx.enter_context(tc.tile_pool(name="wstage", bufs=3))
        big = ctx.enter_context(tc.tile_pool(name="big", bufs=1))
        epool = ctx.enter_context(tc.tile_pool(name="epool", bufs=3))
        rpool = ctx.enter_context(tc.tile_pool(name="rpool", bufs=2))
        opool = ctx.enter_context(tc.tile_pool(name="opool", bufs=6))
        ps512 = ctx.enter_context(tc.tile_pool(name="ps512", bufs=2, space="PSUM"))

        # ---- constants ----
        ident = const.tile([P, P], F32)
        make_identity(nc, ident)
        ones_f = const.tile([P, 2], F32)
        nc.vector.memset(ones_f, 0.5)
        warm = const.tile([P, 2], F32)
        nc.scalar.activation(warm[:], ones_f[:], AF.Exp)
        ones_r = const.tile([P, 2], BF16)
        nc.vector.tensor_copy(out=ones_r[:], in_=ones_f[:])
        # ---- big persistent tensors ----
        xT = big.tile([P, DC, N], BF16)     # x^T: d on partitions
        YT = big.tile([P, DC, N], BF16)     # (xA + v)^T: d' on partitions
        V = big.tile([P, NT, D], BF16)      # natural: seq on partitions
        A_sb = big.tile([P, DC, D], BF16)   # A[d, d'], d = ki*128+p
        wqT = big.tile([P, DC, D], BF16)    # WqT[k, d], k = kc*128+p
        wkT = big.tile([P, DC, D], BF16)
        v_sb = const.tile([P, DC], F32)     # v = Wk bq, chunked like bq_col

        # ---- phase 0: weights (sync queue), x tiles (scalar queue) ----
        wstg = {}
        for name, wap in (("q", wq), ("k", wk), ("v", wv)):
            wst = wstage.tile([P, DC, D], F32, tag="wstage", name=f"wst_{name}")
            wre = wap.rearrange("(dc p) k -> p dc k", p=P)
            for dc in range(DC):
                nc.sync.dma_start(wst[:, dc, :], wre[:, dc, :])
            wstg[name] = wst
        wv_st = wstg["v"]
        bv_rep = const.tile([P, D], F32)
        nc.sync.dma_start(bv_rep[:], bv[None, :].to_broadcast((P, D)))
        bq_col = const.tile([P, DC], F32)
        nc.sync.dma_start(bq_col[:], bq.rearrange("(c p) -> p c", p=P))
        bq_col_r = const.tile([P, DC, 2], BF16)
        nc.vector.tensor_copy(out=bq_col_r[:, :, 0], in_=bq_col[:])
        nc.vector.tensor_copy(out=bq_col_r[:, :, 1], in_=bq_col[:])

        with tc.tile_pool(name="ps_tr", bufs=2, space="PSUM") as ps_tr:
            # transposes: wT[k, d] = W[d, k]; 4 blocks batched per PSUM bank,
            # one strided copy out (dst view [P, kc, 128] at fixed dc)
            for name, wT in (("q", wqT), ("k", wkT)):
                wst = wstg[name]
                for dc in range(DC):
                    tp = ps_tr.tile([P, DC, P], F32, tag="tr")
                    for kc in range(DC):
                        nc.tensor.transpose(tp[:, kc, :], wst[:, dc, ts(kc, P)],
                                            ident)
                    nc.vector.tensor_copy(out=wT[:, :, ts(dc, P)], in_=tp[:])

            def emit_A_v():
                # A[d-chunk dc, d'] = sum_k Wq[d, k] Wk[d', k]
                for dc in range(DC):
                    pa = ps512.tile([P, QS], F32, tag="mm512", name=f"pa_{dc}")
                    for kc in range(DC):
                        nc.tensor.matmul(
                            pa[:], wqT[:, kc, ts(dc, P)], wkT[:, kc, :],
                            start=(kc == 0), stop=(kc == DC - 1),
                        )
                    nc.vector.tensor_copy(out=A_sb[:, dc, :], in_=pa[:])
                # v = Wk @ bq (sequential accumulation groups, one per column)
                pv_ps = ps_tr.tile([P, DC, 2], F32, tag="tr", name="v_ps")
                for dc in range(DC):
                    for jc in range(DC):
                        nc.tensor.matmul(
                            pv_ps[:, dc, :], wkT[:, jc, ts(dc, P)],
                            bq_col_r[:, jc, :],
                            start=(jc == 0), stop=(jc == DC - 1),
                            skip_group_check=True,
                        )
                nc.vector.tensor_copy(out=v_sb[:], in_=pv_ps[:, :, 0])

            wv_r = big.tile([P, DC, D], BF16)
            for dc in range(DC):
                nc.vector.tensor_copy(out=wv_r[:, dc, :], in_=wv_st[:, dc, :])

            # ---- phase 1: x tiles -> xT (PE transpose); V; YT per strip ----
            # Software-pipelined: V(t) and YT(s) are emitted one x-tile late so
            # the PE never head-of-line blocks on the PSUM->SBUF copy of xT.
            def emit_v(t):
                pv = ps512.tile([P, QS], F32, tag="mm512", name=f"pv_{t}")
                for ki in range(DC):
                    nc.tensor.matmul(
                        pv[:], xT[:, ki, ts(t, P)], wv_r[:, ki, :],
                        start=(ki == 0), stop=(ki == DC - 1),
                    )
                nc.vector.tensor_add(out=V[:, t, :], in0=pv[:], in1=bv_rep[:])

            def emit_yt(s):
                for co in range(DC):
                    py = ps512.tile([P, QS], F32, tag="mm512",
                                    name=f"py_{s}_{co}")
                    for ki in range(DC):
                        nc.tensor.matmul(
                            py[:], A_sb[:, ki, ts(co, P)],
                            xT[:, ki, ts(s, QS)],
                            start=(ki == 0), stop=(ki == DC - 1),
                        )
                    nc.scalar.activation(
                        YT[:, co, ts(s, QS)], py[:], AF.Identity,
                        bias=v_sb[:, co:co + 1],
                    )

            for t in range(NT):
                x_t = io512.tile([P, D], F32, tag="io512")
                nc.scalar.dma_start(x_t[:], x[ts(t, P), :])
                tp = ps_tr.tile([P, DC, P], F32, tag="tr")
                for c in range(DC):
                    nc.tensor.transpose(tp[:, c, :], x_t[:, ts(c, P)], ident)
                # single strided copy into xT (ACT casts fp32 psum -> bf16)
                nc.scalar.copy(xT[:, :, ts(t, P)], tp[:])
                if t == 6:
                    emit_A_v()
                if t >= 1:
                    emit_v(t - 1)
                if t in (8, 10, 12):
                    emit_yt((t - 8) // 2)
            emit_v(NT - 1)
            emit_yt(NS - 1)

        # ---- phase 2: attention, one 512-wide q strip at a time ----
        with tc.tile_pool(name="ps_o", bufs=4, space="PSUM") as ps_o, \
             tc.tile_pool(name="ps_sums", bufs=2, space="PSUM") as ps_sums:
            # q-strips: three 512-wide, then two 256-wide so the exposed
            # epilogue after the very last matmul covers only 2 output tiles
            strips = [(0, 4), (4, 4), (8, 4), (12, 2), (14, 2)]
            for si, (q0, nsub) in enumerate(strips):
                qw = nsub * P
                sums_ps = ps_sums.tile([P, DC, NT, 2], F32, tag="sums")
                o_ps = [ps_o.tile([P, QS], F32, tag="o", name=f"o_{si}_{c}")
                        for c in range(nsub)]

                def emit_mo(kt, e):
                    for sub in range(nsub):
                        nc.tensor.matmul(
                            sums_ps[:, sub, kt, :], e[:, ts(sub, P)],
                            ones_r[:], start=True, stop=True,
                        )
                        nc.tensor.matmul(
                            o_ps[sub][:], e[:, ts(sub, P)], V[:, kt, :],
                            start=(kt == 0), stop=(kt == NT - 1),
                            skip_group_check=True,
                        )

                pending = None
                for kt in range(NT):
                    st = ps512.tile([P, QS], F32, tag="mm512",
                                    name=f"st_{si}_{kt}")
                    for ki in range(DC):
                        nc.tensor.matmul(
                            st[:, :qw], xT[:, ki, ts(kt, P)],
                            YT[:, ki, q0 * P:q0 * P + qw],
                            start=(ki == 0), stop=(ki == DC - 1),
                        )
                    e = epool.tile([P, QS], BF16, tag="e")
                    nc.scalar.activation(e[:, :qw], st[:, :qw], AF.Exp,
                                         scale=SOFTMAX_SCALE)
                    if pending is not None:
                        emit_mo(*pending)
                    pending = (kt, e)
                emit_mo(*pending)

                # normalize in natural layout: per-partition scalar multiply
                sums_red = rpool.tile([P, DC], F32, tag="sred")
                nc.vector.tensor_reduce(
                    sums_red[:, :nsub], sums_ps[:, :nsub], axis=mybir.AxisListType.XY,
                    op=mybir.AluOpType.add,
                )
                r = rpool.tile([P, DC], F32, tag="r")
                nc.vector.reciprocal(r[:, :nsub], sums_red[:, :nsub])
                for sub in range(nsub):
                    o_sb = opool.tile([P, D], F32, tag="osb")
                    if sub % 2 == 0:
                        nc.vector.tensor_scalar_mul(o_sb[:], o_ps[sub][:],
                                                    r[:, sub:sub + 1])
                    else:
                        nc.scalar.mul(o_sb[:], o_ps[sub][:],
                                      r[:, sub:sub + 1])
                    dma_eng = nc.sync if sub % 2 == 0 else nc.scalar
                    dma_eng.dma_start(out[ts(q0 + sub, P), :], o_sb[:])

    nc.compile()
    return nc


_CACHE = {}


def _get_nc():
    if "nc" not in _CACHE:
        _CACHE["nc"] = _build()
    return _CACHE["nc"]


def kernel(x, Wq_w, Wq_b, Wk_w, Wk_b, Wv_w, Wv_b, _trace=False, _tmpdir=None):
    nc = _get_nc()
    x = np.ascontiguousarray(np.asarray(x, dtype=np.float32))
    args = {
        "wq": Wq_w, "bq": Wq_b,
        "wk": Wk_w, "bk": Wk_b,
        "wv": Wv_w, "bv": Wv_b,
    }
    args = {k: np.ascontiguousarray(np.asarray(v, dtype=np.float32))
            for k, v in args.items()}
    in_maps = [dict(args, x=x[i]) for i in range(B)]
    res = bass_utils.run_bass_kernel_spmd(
        nc, in_maps, core_ids=list(range(B)),
        trace=_trace, tmpdir=_tmpdir,
    )
    out = np.stack([r["out"] for r in res.results], axis=0)
    if _trace:
        kernel.last_results = res
    return out


if __name__ == "__main__":
    rng = np.random.default_rng(0)
    inputs = {
        "x": rng.standard_normal((B, N, D)).astype(np.float32),
        "Wq_w": (0.02 * rng.standard_normal((D, D))).astype(np.float32),
        "Wq_b": np.zeros(D, np.float32),
        "Wk_w": (0.02 * rng.standard_normal((D, D))).astype(np.float32),
        "Wk_b": np.zeros(D, np.float32),
        "Wv_w": (0.02 * rng.standard_normal((D, D))).astype(np.float32),
        "Wv_b": np.zeros(D, np.float32),
    }
    got = kernel(**inputs)
    print("out shape:", got.shape, got.dtype)
